# revision 35
# baseline (speedup 1.0000x reference)
"""AdditiveAttention distributed Bass kernel for 8 TRN2 NeuronCores (v7).

Data-parallel over batch: B=8 samples -> 1 per core. Weights replicated.

Per-core math (S=2048, D=1024, H=16, HD=64):
  q = X @ W_qv + b_qv                 ; v = q
  alphas = softmax_h((q @ Wq_s + bq_s) * sc)       sc = 1/sqrt(HD)
  gq[d]  = sum_s alphas[s, h(d)] * q[s, d]         h(d) = d // 64
  betas  = softmax_h(((k*gq) @ Wk_s + bk_s) * sc)  k = X @ W_k + b_k
  gk[d]  = gq[d] * sum_s betas[s, h(d)] * k[s, d]
  out = q + (q*gk) @ W_r + b_r

Neither q NOR k is ever materialized -- the only S-sized GEMM is the
single output projection:
  - logits_a = X @ N + ra,   N = W_qv Wq_s [D,16],  ra = b_qv Wq_s + bq_s
  - gq_raw   = W_qv^T (X^T alpha) + b_qv colsum(alpha)   (diag extract)
  - logits_b = X @ M + rb,   M = W_k diag(gq) Wk_s, rb = (b_k*gq) Wk_s + bk_s
  - gk_raw   = W_k^T (X^T beta) + b_k colsum(beta)       (diag extract)
  - out      = X @ W_out + r_out
      W_out = W_qv (I + diag(gk) W_r)         [D,D] x [D,D] product
      r_out = b_qv + (b_qv*gk) @ W_r + b_r
This turns q-GEMM (S D^2) + out-GEMM (S D^2) into W_out (D^3 = half an
S D^2) + out-GEMM: 1/4 less matmul work overall.

Layout: X^T resident as xt (logits stationaries + out-GEMM stationaries),
X natural as xnat (weighted sums).  All matmuls bf16 with f32 PSUM.  Host
pre-casts/pre-tiles everything into SBUF layout (contiguous DMA).  Output
bf16, cast to f32 on host.
"""

import math
import os
from contextlib import ExitStack

import numpy as np

B, S, D, H = 8, 2048, 1024, 16
HD = D // H
SCALE = 1.0 / math.sqrt(HD)
NCORES = 8
P = 128
NDB = D // P      # 8 d-blocks
NSB = S // P      # 16 s-blocks
NCC = D // P      # 8 contraction chunks
SH = 512          # psum free width for big matmuls
NSH = S // SH     # 4
NDH = D // SH     # 2

_CACHE = {}


def _build():
    import concourse.bacc as bacc
    import concourse.tile as tile
    import concourse.mybir as mybir

    f32 = mybir.dt.float32
    bf16 = mybir.dt.bfloat16
    AF = mybir.ActivationFunctionType
    ALU = mybir.AluOpType

    nc = bacc.Bacc("TRN2", target_bir_lowering=False, debug=False,
                   num_devices=NCORES)

    # bf16 inputs, host pre-cast AND pre-tiled into the exact SBUF layout
    # [128, ...] so every bulk DMA is a fully contiguous copy.
    XNt = nc.dram_tensor("XNt", [P, NSB * D], bf16, kind="ExternalInput").ap()
    XTt = nc.dram_tensor("XTt", [P, NCC * S], bf16, kind="ExternalInput").ap()
    W_qv = nc.dram_tensor("Wqvt", [P, NCC * D], bf16, kind="ExternalInput").ap()
    W_qvT = nc.dram_tensor("WqvTt", [P, NDB * D], bf16, kind="ExternalInput").ap()
    W_k = nc.dram_tensor("Wkt", [P, NCC * D], bf16, kind="ExternalInput").ap()
    W_kT = nc.dram_tensor("WkTt", [P, NDB * D], bf16, kind="ExternalInput").ap()
    W_r = nc.dram_tensor("Wrt", [P, NCC * D], bf16, kind="ExternalInput").ap()
    N_t = nc.dram_tensor("Nt", [P, NCC * H], bf16, kind="ExternalInput").ap()
    ra_c = nc.dram_tensor("ra_colt", [16, 1], f32, kind="ExternalInput").ap()
    Wk_s = nc.dram_tensor("Wkst", [P, NDB * H], bf16, kind="ExternalInput").ap()
    bks_c = nc.dram_tensor("bks_colt", [16, 1], f32, kind="ExternalInput").ap()
    br_b = nc.dram_tensor("b_rbf", [D], bf16, kind="ExternalInput").ap()
    bqv_b = nc.dram_tensor("b_qvbf", [D], bf16, kind="ExternalInput").ap()
    bkv_b = nc.dram_tensor("b_kbf", [D], bf16, kind="ExternalInput").ap()
    bqv_t = nc.dram_tensor("b_qvbt", [P, NDB], bf16, kind="ExternalInput").ap()
    bk_t = nc.dram_tensor("b_kbt", [P, NDB], bf16, kind="ExternalInput").ap()
    OUT = nc.dram_tensor("out", [S, D], bf16, kind="ExternalOutput").ap()

    with tile.TileContext(nc) as tc, ExitStack() as ctx:
        sbp = ctx.enter_context(tc.tile_pool(name="sbp", bufs=1))
        psp = ctx.enter_context(tc.tile_pool(name="psp", bufs=1, space="PSUM"))

        def st(shape, dt_, tag, bufs=1):
            return sbp.tile(shape, dt_, tag=tag, bufs=bufs, name=tag)

        def pt_(shape, tag, bufs):
            return psp.tile(shape, f32, tag=tag, bufs=bufs, name=tag)


        # ---------- small persistent intermediates ----------
        aE = st([P, NSB * H], f32, "aE")
        Za = st([P, NSB], f32, "Za")
        rZa = st([P, NSB], f32, "rZa")
        alpha = st([P, NSB * H], bf16, "alpha")
        bEx = st([P, NSB * H], f32, "bEx")
        Zb = st([P, NSB], f32, "Zb")
        rZb = st([P, NSB], f32, "rZb")
        beta = st([P, NSB * H], bf16, "beta")
        Asb_q = st([P, NCC * H], bf16, "Asbq")
        Ssb_q = st([1, H], bf16, "Ssbq")
        Asb_k = st([P, NCC * H], bf16, "Asbk")
        Ssb_k = st([1, H], bf16, "Ssbk")
        gq = st([P, NDB], f32, "gq")
        gkd = st([P, NDB], f32, "gkd")
        gk = st([P, NDB], f32, "gk")
        nks = st([P, NCC * H], bf16, "nks")     # N = W_qv Wq_s (host)
        ra_col = st([16, 1], f32, "ra_col")    # alpha-logits bias (host)
        mt_sb = st([16, D], bf16, "mt_sb")      # M^T  [h, c]
        mks = st([P, NCC * H], bf16, "mks")     # M    [c-part, (cc, h)]
        rb_col = st([16, 1], f32, "rb_col")    # beta-logits bias
        bks_col = st([16, 1], f32, "bks_col")
        lgT_a = st([16, S], bf16, "lgT_a")      # logits^T staging
        lgT_b = st([16, S], bf16, "lgT_b")

        # ---------- big persistent activations / resident data ----------
        xt = st([P, NCC * S], bf16, "xt")   # X^T, chunk cc at cols cc*S
        xnat = st([P, NSB * D], bf16, "xnat")  # natural X, s-block si at si*D
        wqv_all = st([P, NCC * D], bf16, "wqv_all")
        wqvt_all = st([P, NDB * D], bf16, "wqvt_all")  # W_qv^T
        wk_all = st([P, NCC * D], bf16, "wk_all")
        wkt_all = st([P, NDB * D], bf16, "wkt_all")    # W_k^T
        wr_all = st([P, NCC * D], bf16, "wr_all")      # becomes diag(gk) W_r
        wout = st([P, NCC * D], bf16, "wout")          # W_qv (I + diag(gk) W_r)

        # ---------- DMA schedule ----------
        # Phase needs: N <- wqvT (t~10); logits_a <- xt (t~15..25);
        # ws_q A <- xnat (t~28); grawT_q <- wqv (t~38); MT <- wkT (t~45);
        # ws_k <- wk (t~55); fold <- wr (t~60).
        # sync HWDGE and gpsimd SWDGE each sustain ~120-330 GB/s and
        # contend; split the early-critical tensors across both.
        HB = NCC // 2

        def xt_slice(sh, half, eng):
            lo = half * HB
            v = xt[:, lo * S:(lo + HB) * S].rearrange(
                "p (cc s) -> p cc s", cc=HB)[:, :, sh * SH:(sh + 1) * SH]
            s_ = XTt[:, lo * S:(lo + HB) * S].rearrange(
                "p (cc s) -> p cc s", cc=HB)[:, :, sh * SH:(sh + 1) * SH]
            eng.dma_start(v, s_)

        nc.sync.dma_start(wqvt_all[:, :4 * D], W_qvT[:, :4 * D])
        nc.gpsimd.dma_start(wqvt_all[:, 4 * D:], W_qvT[:, 4 * D:])
        for sh in range(NSH):
            xt_slice(sh, 0, nc.sync)
            xt_slice(sh, 1, nc.gpsimd)
        nc.sync.dma_start(xnat[:, :NSB * D // 2], XNt[:, :NSB * D // 2])
        nc.gpsimd.dma_start(xnat[:, NSB * D // 2:], XNt[:, NSB * D // 2:])
        nc.sync.dma_start(wqv_all[:], W_qv)
        nc.gpsimd.dma_start(wkt_all[:], W_kT)
        nc.sync.dma_start(wk_all[:], W_k)
        nc.sync.dma_start(wr_all[:], W_r)

        # ---------- constants ----------
        ones_row = st([1, P], bf16, "ones_row")
        nc.gpsimd.memset(ones_row[:], 1.0)
        ones_col = st([P, 1], bf16, "ones_col")
        nc.gpsimd.memset(ones_col[:], 1.0)
        ones16 = st([16, 1], bf16, "ones16")
        nc.gpsimd.memset(ones16[:], 1.0)
        eye16 = st([16, 16], bf16, "eye16")
        nc.gpsimd.memset(eye16[:], 1.0)
        nc.gpsimd.affine_select(eye16[:], eye16[:], pattern=[[1, 16]],
                                compare_op=ALU.is_equal, fill=0.0,
                                base=0, channel_multiplier=-1)
        # head-selector mask: hmask[h, d] = 1 iff h == d//64   (16 partitions)
        hmask = st([16, D], f32, "hmask")
        hm2 = st([16, D], f32, "hm2")
        nc.gpsimd.memset(hmask[:], 1.0)
        nc.gpsimd.memset(hm2[:], 1.0)
        nc.gpsimd.affine_select(hmask[:], hmask[:], pattern=[[1, D]],
                                compare_op=ALU.is_ge, fill=0.0,
                                base=0, channel_multiplier=-HD)
        nc.gpsimd.affine_select(hm2[:], hm2[:], pattern=[[1, D]],
                                compare_op=ALU.is_ge, fill=0.0,
                                base=-HD, channel_multiplier=-HD)
        nc.vector.tensor_sub(hmask[:], hmask[:], hm2[:])

        # ---------- small loads (scalar HWDGE queue) ----------
        nc.scalar.dma_start(nks[:], N_t)
        nc.scalar.dma_start(ra_col[:], ra_c)
        nc.scalar.dma_start(bks_col[:], bks_c)
        bqv_bf = st([P, NDB], bf16, "bqv_bf")
        nc.scalar.dma_start(bqv_bf[:], bqv_t)
        wks_sb = st([P, NDB * H], bf16, "wks_sb")   # becomes diag(gq)-scaled
        nc.scalar.dma_start(wks_sb[:], Wk_s)
        bk_bf = st([P, NDB], bf16, "bk_bf")
        nc.scalar.dma_start(bk_bf[:], bk_t)
        bqv_row = st([1, D], bf16, "bqv_row")
        nc.scalar.dma_start(bqv_row[:], bqv_b.unsqueeze(0))
        bk_row = st([1, D], bf16, "bk_row")
        nc.scalar.dma_start(bk_row[:], bkv_b.unsqueeze(0))
        br_row = st([1, D], bf16, "br_row")
        nc.scalar.dma_start(br_row[:], br_b.unsqueeze(0))

        # ---------- generic [D,16] = W' diag(s) Ws fold + transpose ----------
        # rT[h, c] = sum_j ws[j-block]^T @ WT[j-block, c]; transpose -> [c,h]
        def fold16(ws_col, wT, t_sb, out_ks):
            t_ps = pt_([16, D], "grawT", 1)
            for dh in range(NDH):
                for j in range(NDB):
                    nc.tensor.matmul(
                        t_ps[:, dh * SH:(dh + 1) * SH],
                        ws_col[:, j * H:(j + 1) * H],
                        wT[:, j * D + dh * SH: j * D + dh * SH + SH],
                        start=(j == 0), stop=(j == NDB - 1))
                nc.vector.tensor_copy(t_sb[:, dh * SH:(dh + 1) * SH],
                                      t_ps[:, dh * SH:(dh + 1) * SH])
            m_ps = pt_([P, NCC * H], "small", 3)
            for cc in range(NCC):
                nc.tensor.matmul(
                    m_ps[:, cc * H:(cc + 1) * H],
                    t_sb[:, cc * P:(cc + 1) * P], eye16[:, :],
                    start=True, stop=True)
            nc.vector.tensor_copy(out_ks[:], m_ps[:])

        # bias fold: r[h] = sum_d bcol[d] ws[d, h] + badd[h]   (column out)
        def foldbias(bcol, ws_col, badd_col, out_col):
            r_ps = pt_([16, 1], "small", 3)
            for j in range(NDB):
                nc.tensor.matmul(
                    r_ps[:, :1], ws_col[:, j * H:(j + 1) * H],
                    bcol[:, j:j + 1],
                    start=(j == 0), stop=(j == NDB - 1))
            nc.vector.tensor_tensor(out_col[:16, :1], r_ps[:, :1],
                                    badd_col[:16, :1], ALU.add)

        # ---------- logits + softmax (alphas / betas) ----------
        # logits^T[h, s] via 512-wide chains (vs 144 LDW-bound 16-wide
        # matmuls), bias fused into the psum->sbuf copy, then 16 cheap PE
        # transposes back to [s, h] for the partitionwise softmax.
        def softmax_weights(w16, r_col, lgT_sb, eE, Z, rZ, wout_):
            for sh in range(NSH):
                lp = pt_([16, SH], "lgT", 1)
                for cc in range(NCC):
                    nc.tensor.matmul(
                        lp[:], w16[:, cc * H:(cc + 1) * H],
                        xt[:, cc * S + sh * SH: cc * S + sh * SH + SH],
                        start=(cc == 0), stop=(cc == NCC - 1))
                nc.vector.tensor_scalar(
                    lgT_sb[:, sh * SH:(sh + 1) * SH], lp[:],
                    r_col[:16, 0:1], None, ALU.add)
            HS = NSB // 2
            for hf in range(2):
                lg = pt_([P, HS * H], "small", 3)
                sb0 = hf * HS
                for sb in range(sb0, sb0 + HS):
                    c = (sb - sb0) * H
                    nc.tensor.matmul(
                        lg[:, c:c + H],
                        lgT_sb[:, sb * P:(sb + 1) * P], eye16[:, :],
                        start=True, stop=True)
                cs = sb0 * H
                nc.scalar.activation(eE[:, cs:cs + HS * H], lg[:],
                                     AF.Exp, bias=0.0, scale=SCALE)
                nc.vector.reduce_sum(
                    Z[:, sb0:sb0 + HS].unsqueeze(2),
                    eE[:, cs:cs + HS * H].rearrange("p (sb h) -> p sb h", sb=HS),
                    axis=mybir.AxisListType.X)
                nc.vector.reciprocal(rZ[:, sb0:sb0 + HS], Z[:, sb0:sb0 + HS])
                nc.vector.tensor_tensor(
                    wout_[:, cs:cs + HS * H].rearrange("p (sb h) -> p sb h", sb=HS),
                    eE[:, cs:cs + HS * H].rearrange("p (sb h) -> p sb h", sb=HS),
                    rZ[:, sb0:sb0 + HS].unsqueeze(2).broadcast_to([P, HS, H]),
                    ALU.mult)

        # ---------- g_raw = W^T (X^T w) + b colsum(w), extract diagonal ----------
        def weighted_sum(weights_sb, wall, b_row, Asb, Ssb, g):
            Aps = pt_([P, NCC * H], "small", 3)
            Sps = pt_([1, H], "small", 3)
            for si in range(NSB):
                for cb in range(NCC):
                    nc.tensor.matmul(
                        Aps[:, cb * H:(cb + 1) * H],
                        xnat[:, si * D + cb * P: si * D + cb * P + P],
                        weights_sb[:, si * H:(si + 1) * H],
                        start=(si == 0 and cb == 0),
                        stop=(si == NSB - 1 and cb == NCC - 1))
                nc.tensor.matmul(
                    Sps[:1, :], ones_col[:, :1],
                    weights_sb[:, si * H:(si + 1) * H],
                    start=(si == 0), stop=(si == NSB - 1))
            nc.vector.tensor_copy(Asb[:], Aps[:])
            nc.vector.tensor_copy(Ssb[:1, :], Sps[:1, :])
            # g_rawT[h, d] = (A^T W)[h, d] + S[h] b[d]   (16-col stationary)
            grawT = pt_([16, D], "grawT", 1)
            for dh in range(NDH):
                for cc in range(NCC):
                    nc.tensor.matmul(
                        grawT[:, dh * SH:(dh + 1) * SH],
                        Asb[:, cc * H:(cc + 1) * H],
                        wall[:, cc * D + dh * SH: cc * D + dh * SH + SH],
                        start=(cc == 0), stop=False)
                nc.tensor.matmul(
                    grawT[:, dh * SH:(dh + 1) * SH],
                    Ssb[:1, :], b_row[:1, dh * SH:(dh + 1) * SH],
                    start=False, stop=True)
            # diagonal extract: g[d] = g_rawT[d//64, d]  via mask + ones-matmul
            msk = st([16, D], bf16, "msk", bufs=2)
            nc.vector.tensor_tensor(msk[:], grawT[:], hmask[:], ALU.mult)
            gps = pt_([P, NDB], "small", 3)
            for j in range(NDB):
                nc.tensor.matmul(
                    gps[:, j:j + 1],
                    msk[:, j * P:(j + 1) * P], ones16[:, :1],
                    start=(j == 0), stop=(j == NDB - 1))
            nc.vector.tensor_copy(g[:], gps[:])

        # ---------- alphas:  logits_a = X @ N + ra  (N, ra host-folded) ----------
        softmax_weights(nks, ra_col, lgT_a, aE, Za, rZa, alpha)
        # gq
        weighted_sum(alpha, wqv_all, bqv_row, Asb_q, Ssb_q, gq)
        # fold gq into Wk_s:  wks_sb <- diag(gq) Wk_s
        for j in range(NDB):
            nc.vector.tensor_scalar(
                wks_sb[:, j * H:(j + 1) * H], wks_sb[:, j * H:(j + 1) * H],
                gq[:, j:j + 1], None, ALU.mult)
        # betas:  logits_b = X @ M + rb,  M = W_k diag(gq) Wk_s
        foldbias(bk_bf, wks_sb, bks_col, rb_col)
        fold16(wks_sb, wkt_all, mt_sb, mks)
        softmax_weights(mks, rb_col, lgT_b, bEx, Zb, rZb, beta)
        # gk = gq * diag(W_k^T (X^T beta) + b_k colsum(beta))
        weighted_sum(beta, wk_all, bk_row, Asb_k, Ssb_k, gkd)
        nc.vector.tensor_mul(gk[:], gq[:], gkd[:])

        # ---------- T = diag(gk) W_r  (in place on wr_all) ----------
        for dh in range(NDH):
            for cc in range(NCC):
                lo = cc * D + dh * SH
                nc.vector.tensor_scalar(
                    wr_all[:, lo:lo + SH], wr_all[:, lo:lo + SH],
                    gk[:, cc:cc + 1], None, ALU.mult)

        # ---------- r_out = b_qv + b_qv^T (diag(gk) W_r) + b_r, broadcast ----------
        br_bc = st([P, D], bf16, "br_bc")
        for dh in range(NDH):
            r2 = pt_([1, SH], "small", 3)
            for cc in range(NCC):
                nc.tensor.matmul(
                    r2[:1, :], bqv_bf[:, cc:cc + 1],
                    wr_all[:, cc * D + dh * SH: cc * D + dh * SH + SH],
                    start=(cc == 0), stop=(cc == NCC - 1))
            rrow = st([1, SH], bf16, "rrow", bufs=2)
            nc.vector.tensor_tensor(rrow[:1, :], r2[:1, :],
                                    bqv_row[:1, dh * SH:(dh + 1) * SH], ALU.add)
            nc.vector.tensor_add(rrow[:1, :], rrow[:1, :],
                                 br_row[:1, dh * SH:(dh + 1) * SH])
            bps = pt_([P, SH], "big", 2)
            nc.tensor.matmul(bps[:], ones_row[:1, :], rrow[:1, :],
                             start=True, stop=True)
            nc.vector.tensor_copy(br_bc[:, dh * SH:(dh + 1) * SH], bps[:])

        # ---------- W_out = W_qv @ (I + T) = W_qv + W_qv @ T ----------
        for dh in range(NDH):
            for cb in range(NCC):
                ps = pt_([P, SH], "big", 2)
                for j in range(NDB):
                    nc.tensor.matmul(
                        ps[:], wqvt_all[:, j * D + cb * P: j * D + cb * P + P],
                        wr_all[:, j * D + dh * SH: j * D + dh * SH + SH],
                        start=(j == 0), stop=(j == NDB - 1))
                lo = cb * D + dh * SH
                nc.vector.tensor_tensor(
                    wout[:, lo:lo + SH], ps[:], wqv_all[:, lo:lo + SH],
                    ALU.add)

            # ---------- out tiles for this dh:  out = X @ W_out + r_out ----------
            for sb in range(NSB):
                ps = pt_([P, SH], "big", 2)
                for cc in range(NCC):
                    nc.tensor.matmul(
                        ps[:], xt[:, cc * S + sb * P: cc * S + sb * P + P],
                        wout[:, cc * D + dh * SH: cc * D + dh * SH + SH],
                        start=(cc == 0), stop=(cc == NCC - 1))
                ob = st([P, SH], bf16, "ob", bufs=3)
                nc.vector.tensor_tensor(
                    ob[:], ps[:], br_bc[:, dh * SH:(dh + 1) * SH], ALU.add)
                eng = nc.sync if sb % 2 == 0 else nc.scalar
                eng.dma_start(
                    OUT[sb * P:(sb + 1) * P, dh * SH:(dh + 1) * SH], ob[:])

    nc.compile()
    return nc


def _get_nc():
    if "nc" not in _CACHE:
        _CACHE["nc"] = _build()
    return _CACHE["nc"]


def _tile_rows(a, p=P):
    """[N*p, M] -> [p, N*M] matching the SBUF layout (block n at cols n*M)."""
    n = a.shape[0] // p
    return np.ascontiguousarray(
        a.reshape(n, p, a.shape[1]).transpose(1, 0, 2).reshape(p, -1))


def _prep_inputs(inputs):
    import ml_dtypes
    bf = ml_dtypes.bfloat16

    def f32a(k):
        return np.asarray(inputs[k], dtype=np.float32)

    def c(a):
        return np.ascontiguousarray(np.asarray(a, dtype=np.float32).astype(bf))

    common = {
        "Wqvt": c(_tile_rows(f32a("W_qv"))),
        "WqvTt": c(_tile_rows(np.ascontiguousarray(f32a("W_qv").T))),
        "Wkt": c(_tile_rows(f32a("W_k"))),
        "WkTt": c(_tile_rows(np.ascontiguousarray(f32a("W_k").T))),
        "Wrt": c(_tile_rows(f32a("W_r"))),
        "Nt": c(_tile_rows(f32a("W_qv") @ f32a("Wq_s"))),
        "ra_colt": np.ascontiguousarray(
            (f32a("b_qv") @ f32a("Wq_s") + f32a("bq_s")).reshape(16, 1)),
        "Wkst": c(_tile_rows(f32a("Wk_s"))),
        "bks_colt": np.ascontiguousarray(f32a("bk_s").reshape(16, 1)),
        "b_rbf": c(inputs["b_r"]), "b_qvbf": c(inputs["b_qv"]),
        "b_kbf": c(inputs["b_k"]),
        "b_qvbt": c(f32a("b_qv").reshape(NDB, P).T),
        "b_kbt": c(f32a("b_k").reshape(NDB, P).T),
    }
    in_maps = []
    for b in range(NCORES):
        m = dict(common)
        xb = np.asarray(inputs["X"][b], dtype=np.float32)
        m["XNt"] = c(_tile_rows(xb))
        m["XTt"] = c(_tile_rows(np.ascontiguousarray(xb.T)))
        in_maps.append(m)
    return in_maps


def run(inputs, trace=False):
    from concourse.bass_utils import run_bass_kernel_spmd

    nc = _get_nc()
    in_maps = _prep_inputs(inputs)
    res = run_bass_kernel_spmd(nc, in_maps, core_ids=list(range(NCORES)),
                               trace=trace)
    _CACHE["last_results"] = res
    out = np.stack([np.asarray(res.results[b]["out"], dtype=np.float32)
                    for b in range(NCORES)], axis=0)
    return out


def kernel(**inputs):
    trace = os.environ.get("KTRACE", "0") == "1"
    return run(inputs, trace=trace)
